# revision 13
# baseline (speedup 1.0000x reference)
"""Trainium2 Bass kernel for the KAN-to-MLP module.

Math: out = GELU( silu(x) @ base_w.T + einsum('nhk,ohk->no', bsplines(x), spline_w * scaler) )

Both branches fuse into ONE PSUM accumulation per output tile with
contraction K = H (silu branch, bf16) + 8*H (B-spline planes, fp8
DoubleRow).  The uniform cubic B-spline bases are computed on-device in
closed form: for u = 2.5x + 2.5, i = floor(u), t = u - i, the only
nonzero bases are planes j = i..i+3 with values
[(1-t)^3/6, (3t^3-6t^2+4)/6, (-3t^3+3t^2+3t+1)/6, t^3/6].

Precision plan (validated vs reference, rel err ~8.6e-3):
  - x shipped fp16; u, t and the is_ge/floor chain in fp32 (fp16 u has
    too coarse an ulp near u=5); basis values stored fp16.
  - spline weights hosted-packed to fp8e4 scaled by SW=64; basis planes
    cast fp16->fp8 on ACT with scale SF=16; base weights bf16 scaled by
    SW*SF so one PSUM group accumulates both branches; GELU applied on
    ACT with input scale 1/(SW*SF).
  - fp8 pairs feed MatmulPerfMode.DoubleRow (2 k-tiles/instruction,
    ~3x bf16 throughput measured on HW).

Sharding: data-parallel over tokens (8192 rows -> 1024/core), weights
replicated.  Per core tokens are processed in 2 chunks of 512 so the
feature build (DVE/ACT) of chunk c+1 overlaps the matmul sweep of
chunk c; weights stream from HBM once per chunk.
"""

import sys

for _p in ("/opt/trn_rl_repo",):
    if _p not in sys.path:
        sys.path.insert(0, _p)

import numpy as np
import ml_dtypes

import concourse.bass as bass
import concourse.tile as tile
from concourse import bacc, mybir
from concourse.bass_utils import run_bass_kernel_spmd

AF = mybir.ActivationFunctionType
ALU = mybir.AluOpType
DT = mybir.dt
PM = mybir.MatmulPerfMode

N_CORES = 8
NTOK = 1024          # tokens per core
H = 1024             # input dim
D = 4096             # output dim
NB = 8               # number of basis planes
CHUNK = 512          # tokens per chunk
NCHUNK = NTOK // CHUNK
DTI = D // 128       # 32 output tiles
HT = H // 128        # 8 h-tiles
NJP = NB // 2        # 4 DoubleRow plane-pairs per h-tile

SW = 64.0            # spline weight scale (fp8 dynamic-range placement)
SF = 16.0            # basis feature scale at the fp16->fp8 cast
CL = 4.9999995       # clamp below 5 so floor(u) <= 4

_NC_CACHE = {}


def _emit_workload(nc, tc, pools, xt, wb, w8, out):
    """One full per-core workload: all chunks, feature build + matmul."""
    f32, f16, bf16, f8 = DT.float32, DT.float16, DT.bfloat16, DT.float8e4
    xp, scr, bp, fp16p, fp8p, silup, wbp, w8p, psump, outp = pools

    all_silu = []
    all_fp8 = []
    for c in range(NCHUNK):
        # ---- load x chunk: [128, ht, tok] fp16, one DMA ----
        xc = xp.tile([128, HT, CHUNK], f16, tag="xc", name="xc")
        nc.sync.dma_start(xc[:], xt[:, :, c * CHUNK:(c + 1) * CHUNK])

        silu_t = []
        fp8_t = []
        for ht in range(HT):
            xs = xc[:, ht, :]
            # silu branch feature (bf16)
            st = silup.tile([128, CHUNK], bf16, tag=f"sl{ht}", name="sl")
            nc.scalar.activation(st[:], xs, AF.Silu)
            silu_t.append(st)

            # u = 2.5x + 2.5 in fp32, clamped below 5
            u = scr.tile([128, CHUNK], f32, tag="u", name="u")
            nc.scalar.activation(u[:], xs, AF.Copy, bias=2.5, scale=2.5)
            uc = scr.tile([128, CHUNK], f32, tag="uc", name="uc")
            nc.vector.tensor_scalar_min(uc[:], u[:], CL)
            # floor(u) as sum of fp16 step functions (exact small ints)
            g = []
            for v in range(1, 5):
                gv = scr.tile([128, CHUNK], f16, tag=f"g{v}", name="g")
                nc.vector.tensor_scalar(gv[:], uc[:], float(v), None,
                                        ALU.is_ge)
                g.append(gv)
            i12 = scr.tile([128, CHUNK], f16, tag="i12", name="i12")
            nc.vector.tensor_add(i12[:], g[0][:], g[1][:])
            i34 = scr.tile([128, CHUNK], f16, tag="i34", name="i34")
            nc.vector.tensor_add(i34[:], g[2][:], g[3][:])
            ii = bp.tile([128, CHUNK], f16, tag="ii", name="ii")
            nc.vector.tensor_add(ii[:], i12[:], i34[:])
            # t = u - floor(u) in fp32 (fp16 u has too coarse an ulp), then
            # fp16 powers; bases are scaled by 6 (folded into the fp8 cast)
            t = scr.tile([128, CHUNK], f32, tag="t", name="t")
            nc.vector.tensor_sub(t[:], uc[:], ii[:])
            t16 = bp.tile([128, CHUNK], f16, tag="t16", name="t16")
            nc.vector.tensor_copy(t16[:], t[:])
            t2 = bp.tile([128, CHUNK], f16, tag="t2", name="t2")
            nc.scalar.activation(t2[:], t[:], AF.Square)
            t3 = bp.tile([128, CHUNK], f16, tag="t3", name="t3")
            nc.vector.tensor_mul(t3[:], t2[:], t16[:])
            s = bp.tile([128, CHUNK], f16, tag="s", name="s")
            nc.vector.tensor_scalar(s[:], t16[:], -1.0, 1.0,
                                    ALU.mult, ALU.add)
            s2 = bp.tile([128, CHUNK], f16, tag="s2", name="s2")
            nc.scalar.activation(s2[:], t[:], AF.Square, scale=-1.0, bias=1.0)
            # B_d = 6*b_d:  B0=(1-t)^3  B1=3t^3-6t^2+4  B2=6-B0-B1-B3  B3=t^3
            B0 = bp.tile([128, CHUNK], f16, tag="B0", name="B0")
            nc.vector.tensor_mul(B0[:], s2[:], s[:])
            B3 = t3
            d2 = scr.tile([128, CHUNK], f16, tag="d2", name="d2")
            nc.vector.tensor_scalar_mul(d2[:], t2[:], 2.0)
            qq = scr.tile([128, CHUNK], f16, tag="qq", name="qq")
            nc.vector.tensor_sub(qq[:], t3[:], d2[:])
            B1 = bp.tile([128, CHUNK], f16, tag="B1", name="B1")
            nc.vector.tensor_scalar(B1[:], qq[:], 3.0, 4.0,
                                    ALU.mult, ALU.add)
            B2a = scr.tile([128, CHUNK], f16, tag="B2a", name="B2a")
            nc.vector.tensor_add(B2a[:], B0[:], B3[:])
            B2b = scr.tile([128, CHUNK], f16, tag="B2b", name="B2b")
            nc.vector.tensor_add(B2b[:], B2a[:], B1[:])
            B2 = bp.tile([128, CHUNK], f16, tag="B2", name="B2")
            nc.vector.tensor_scalar(B2[:], B2b[:], -1.0, 6.0,
                                    ALU.mult, ALU.add)
            bd = (B0, B1, B2, B3)
            # interval masks (tensor_scalar: 4x DVE mode, unlike the 1x
            # scalar_tensor_tensor path)
            masks = []
            for iv in range(5):
                mk = bp.tile([128, CHUNK], f16, tag=f"mk{iv}", name="mk")
                nc.vector.tensor_scalar(mk[:], ii[:], float(iv), None,
                                        ALU.is_equal)
                masks.append(mk)

            # plane j = sum_d mask_{j-d} * B_d, paired (2jp, 2jp+1) in one
            # fp16 tile then cast *(SF/6) to one fp8 DoubleRow tile on ACT
            fp8_ht = []
            for jp in range(NJP):
                pair16 = fp16p.tile([128, 2 * CHUNK], f16, tag=f"pr{jp % 2}",
                                    name="pair16")
                for half in range(2):
                    j = 2 * jp + half
                    dst = pair16[:, half * CHUNK:(half + 1) * CHUNK]
                    terms = [(j - d, d) for d in range(4) if 0 <= j - d <= 4]
                    if len(terms) == 1:
                        iv, d = terms[0]
                        nc.vector.tensor_mul(dst, masks[iv][:], bd[d][:])
                    else:
                        acc = scr.tile([128, CHUNK], f16, tag="acc",
                                      name="acc")
                        iv, d = terms[0]
                        nc.vector.tensor_mul(acc[:], masks[iv][:], bd[d][:])
                        for n, (iv, d) in enumerate(terms[1:]):
                            last = n == len(terms) - 2
                            tgt = dst if last else acc[:]
                            tmp = scr.tile([128, CHUNK], f16, tag="tmp",
                                          name="tmp")
                            nc.vector.tensor_mul(tmp[:], masks[iv][:],
                                                 bd[d][:])
                            nc.vector.tensor_add(tgt, acc[:], tmp[:])
                pair8 = fp8p.tile([128, 2 * CHUNK], f8,
                                  tag=f"f8_{ht}_{jp}", name="pair8")
                nc.scalar.activation(pair8[:], pair16[:], AF.Copy,
                                     scale=SF / 6.0)
                fp8_ht.append(pair8)
            fp8_t.append(fp8_ht)
        all_silu.append(silu_t)
        all_fp8.append(fp8_t)

    # ---- matmul sweep: ONE weight pass; per di, one PSUM group per
    # chunk reusing the loaded weight tile.  DMAs are batched (2 output
    # tiles per weight DMA, 4 per out DMA) because each 128-partition
    # dma_start costs ~2.4us of descriptor emission on the issuing
    # sequencer.  ----
    NWB = 2              # output tiles per weight DMA
    NOUT = 2             # output tiles per out DMA
    ot4 = {0: None, 1: None}
    for di in range(DTI):
        if di % NWB == 0:
            wbt2 = wbp.tile([128, NWB, HT * 128], bf16, tag="wb",
                            name="wbt2")
            nc.sync.dma_start(
                wbt2[:], wb[di:di + NWB].rearrange("d p c -> p d c"))
            w8t2 = w8p.tile([128, NWB, HT * NJP * 256], f8, tag="w8",
                            name="w8t2")
            nc.sync.dma_start(
                w8t2[:], w8[di:di + NWB].rearrange("d p c -> p d c"))
        dd = di % NWB
        wbt = wbt2[:, dd, :]
        w8t = w8t2[:, dd, :]
        for c in range(NCHUNK):
            silu_t, fp8_t = all_silu[c], all_fp8[c]
            ps = psump.tile([128, CHUNK], f32, tag=f"ps{c}", name="ps")
            for ht in range(HT):
                nc.tensor.matmul(ps[:], wbt[:, ht * 128:(ht + 1) * 128],
                                 silu_t[ht][:], start=(ht == 0), stop=False)
            for ht in range(HT):
                for jp in range(NJP):
                    k = ht * NJP + jp
                    lhsT = w8t[:, k * 256:(k + 1) * 256].rearrange(
                        "p (two m) -> p two m", two=2)
                    rhs = fp8_t[ht][jp][:].rearrange(
                        "p (two c) -> p two c", two=2)
                    nc.tensor.matmul(ps[:], lhsT, rhs,
                                     start=False,
                                     stop=(k == HT * NJP - 1),
                                     perf_mode=PM.DoubleRow)
            oi = di % NOUT
            if oi == 0:
                ot4[c] = outp.tile([128, NOUT, CHUNK], bf16, tag=f"ot{c}",
                                   name="ot4")
            nc.scalar.activation(ot4[c][:, oi, :], ps[:], AF.Gelu,
                                 scale=1.0 / (SW * SF))
            if oi == NOUT - 1:
                d0 = di - (NOUT - 1)
                nc.scalar.dma_start(
                    out[d0 * 128:(d0 + NOUT) * 128,
                        c * CHUNK:(c + 1) * CHUNK].rearrange(
                        "(f p) t -> p f t", f=NOUT),
                    ot4[c][:])


def _build_program(repeat=1):
    nc = bacc.Bacc("TRN2", target_bir_lowering=False, debug=False,
                   enable_asserts=False, num_devices=N_CORES)
    xt = nc.dram_tensor("xt", (128, HT, NTOK), DT.float16,
                        kind="ExternalInput").ap()
    wb = nc.dram_tensor("wb", (DTI, 128, HT * 128), DT.bfloat16,
                        kind="ExternalInput").ap()
    w8 = nc.dram_tensor("w8", (DTI, 128, HT * NJP * 256), DT.float8e4,
                        kind="ExternalInput").ap()
    out = nc.dram_tensor("out", (D, NTOK), DT.bfloat16,
                         kind="ExternalOutput").ap()

    with tile.TileContext(nc) as tc:
        with (
            tc.tile_pool(name="xp", bufs=1) as xp,
            tc.tile_pool(name="scr", bufs=1) as scr,
            tc.tile_pool(name="bp", bufs=2) as bp,
            tc.tile_pool(name="fp16p", bufs=1) as fp16p,
            tc.tile_pool(name="fp8p", bufs=2) as fp8p,
            tc.tile_pool(name="silup", bufs=2) as silup,
            tc.tile_pool(name="wbp", bufs=3) as wbp,
            tc.tile_pool(name="w8p", bufs=3) as w8p,
            tc.tile_pool(name="psump", bufs=4,
                         space=bass.MemorySpace.PSUM) as psump,
            tc.tile_pool(name="outp", bufs=2) as outp,
        ):
            pools = (xp, scr, bp, fp16p, fp8p, silup, wbp, w8p, psump, outp)
            if repeat == 1:
                _emit_workload(nc, tc, pools, xt, wb, w8, out)
            else:
                with tc.For_i(0, repeat, 1) as _:
                    _emit_workload(nc, tc, pools, xt, wb, w8, out)

    nc.compile()
    return nc


def _prep_weights(base_weight, spline_weight, spline_scaler):
    # bf16 block: [di, kk, ht*128+m] = base_w[di*128+m, ht*128+kk] * SW*SF
    wbf = (base_weight.astype(np.float64) * (SW * SF))
    wbf = wbf.reshape(DTI, 128, HT, 128).transpose(0, 3, 2, 1) \
             .reshape(DTI, 128, HT * 128)
    wbf = np.ascontiguousarray(wbf).astype(ml_dtypes.bfloat16)
    # fp8 block: [di, kk, ((ht*NJP+jp)*2+pp)*128+m] =
    #            (spline_w*scaler)[di*128+m, ht*128+kk, 2jp+pp] * SW
    wsp = (spline_weight.astype(np.float64)
           * spline_scaler.astype(np.float64)[..., None]) * SW
    np.clip(wsp, -240.0, 240.0, out=wsp)
    wsp = wsp.reshape(DTI, 128, HT, 128, NJP, 2).transpose(0, 3, 2, 4, 5, 1) \
             .reshape(DTI, 128, HT * NJP * 256)
    wsp = np.ascontiguousarray(wsp).astype(ml_dtypes.float8_e4m3)
    return wbf, wsp


def _prep_x(x):
    xf = np.asarray(x).reshape(N_CORES * NTOK, H).astype(np.float16)
    per_core = []
    for c in range(N_CORES):
        xs = xf[c * NTOK:(c + 1) * NTOK].T          # (H, NTOK)
        xs = np.ascontiguousarray(xs).reshape(HT, 128, NTOK) \
               .transpose(1, 0, 2)                  # (128, HT, NTOK)
        per_core.append(np.ascontiguousarray(xs))
    return per_core


def kernel(x, base_weight, spline_weight, spline_scaler, _trace=False):
    if "nc" not in _NC_CACHE:
        _NC_CACHE["nc"] = _build_program()
    nc = _NC_CACHE["nc"]

    wbf, wsp = _prep_weights(np.asarray(base_weight, np.float32),
                             np.asarray(spline_weight, np.float32),
                             np.asarray(spline_scaler, np.float32))
    xs = _prep_x(x)
    in_maps = [{"xt": xs[c], "wb": wbf, "w8": wsp} for c in range(N_CORES)]

    def run_once():
        res = run_bass_kernel_spmd(nc, in_maps,
                                   core_ids=list(range(N_CORES)),
                                   trace=_trace)
        full = np.concatenate(
            [res.results[c]["out"].astype(np.float32)
             for c in range(N_CORES)], axis=1)      # (4096, 8192)
        if _trace:
            kernel.last_exec_time_ns = res.exec_time_ns
            kernel.last_results = res
        return full

    # Execute twice and compare: very rare transient bad executions have
    # been observed on the first dispatch after device bring-up; a cheap
    # re-execution catches them (results are bit-deterministic when sane).
    full = run_once()
    check = run_once()
    if not np.array_equal(full, check):
        redo = run_once()
        full = check if np.array_equal(check, redo) else redo

    out = np.ascontiguousarray(full.T).reshape(x.shape[0], x.shape[1], D)
    return out.astype(np.float32, copy=False)


def measure_exec_ns(inputs, n=8, repeat_hi=51):
    """Steady-state HW time of one full workload, measured as the
    marginal cost of extra hardware-loop iterations of the whole
    program: (T(repeat_hi) - T(1)) / (repeat_hi - 1).  This cancels the
    (tens of ms, high-variance) axon dispatch overhead that would
    otherwise dominate wall-clock timing."""
    import time
    import jax
    from jax.sharding import Mesh, PartitionSpec, NamedSharding
    try:
        from jax.experimental.shard_map import shard_map
    except ImportError:
        from jax.shard_map import shard_map
    from concourse.bass2jax import (_bass_exec_p, install_neuronx_cc_hook,
                                    partition_id_tensor)

    install_neuronx_cc_hook()

    wbf, wsp = _prep_weights(np.asarray(inputs["base_weight"], np.float32),
                             np.asarray(inputs["spline_weight"], np.float32),
                             np.asarray(inputs["spline_scaler"], np.float32))
    xs = _prep_x(inputs["x"])
    per_core = {"xt": xs, "wb": [wbf] * N_CORES, "w8": [wsp] * N_CORES}

    def timed(repeat):
        key = f"nc{repeat}"
        if key not in _NC_CACHE:
            _NC_CACHE[key] = _build_program(repeat=repeat)
        nc = _NC_CACHE[key]
        pname = (nc.partition_id_tensor.name if nc.partition_id_tensor
                 else None)
        in_names, out_names, out_avals, zero_outs = [], [], [], []
        for alloc in nc.m.functions[0].allocations:
            if not isinstance(alloc, mybir.MemoryLocationSet):
                continue
            name = alloc.memorylocations[0].name
            if alloc.kind == "ExternalInput":
                if name != pname:
                    in_names.append(name)
            elif alloc.kind == "ExternalOutput":
                out_names.append(name)
                shape = tuple(alloc.tensor_shape)
                dtype = mybir.dt.np(alloc.dtype)
                out_avals.append(jax.core.ShapedArray(shape, dtype))
                zero_outs.append(np.zeros(shape, dtype))
        all_in = in_names + out_names + ([pname] if pname else [])

        def _body(*args):
            operands = list(args)
            if pname is not None:
                operands.append(partition_id_tensor())
            outs = _bass_exec_p.bind(
                *operands, out_avals=tuple(out_avals),
                in_names=tuple(all_in), out_names=tuple(out_names),
                lowering_input_output_aliases=(),
                sim_require_finite=True, sim_require_nnan=True, nc=nc)
            return tuple(outs)

        devices = jax.devices()[:N_CORES]
        mesh = Mesh(np.asarray(devices), ("core",))
        sh = NamedSharding(mesh, PartitionSpec("core"))
        fn = jax.jit(shard_map(
            _body, mesh=mesh,
            in_specs=(PartitionSpec("core"),) * (len(in_names)
                                                 + len(out_names)),
            out_specs=(PartitionSpec("core"),) * len(out_names),
            check_rep=False), keep_unused=True)
        concat_in = [jax.device_put(
            np.concatenate(per_core[name], axis=0)
            if isinstance(per_core[name], list)
            else np.concatenate([per_core[name]] * N_CORES, axis=0), sh)
            for name in in_names]
        zeros = [jax.device_put(
            np.zeros((N_CORES * z.shape[0], *z.shape[1:]), z.dtype), sh)
            for z in zero_outs]
        for a in concat_in + zeros:
            a.block_until_ready()
        times = []
        for trial in range(n):
            t0 = time.perf_counter()
            outs = fn(*concat_in, *zeros)
            for o in outs:
                o.block_until_ready()
            dt_s = time.perf_counter() - t0
            if trial > 0:
                times.append(dt_s)
        print(f"  [repeat={repeat}] per-call ms:",
              [f"{t*1e3:.2f}" for t in times])
        return min(times)

    t_lo = timed(1)
    t_hi = timed(repeat_hi)
    return int((t_hi - t_lo) / (repeat_hi - 1) * 1e9)


# revision 14
# speedup vs baseline: 1.1726x; 1.1726x over previous
"""Trainium2 Bass kernel for the KAN-to-MLP module.

Math: out = GELU( silu(x) @ base_w.T + einsum('nhk,ohk->no', bsplines(x), spline_w * scaler) )

Both branches fuse into ONE PSUM accumulation per output tile with
contraction K = H (silu branch, bf16) + 8*H (B-spline planes, fp8
DoubleRow).  The uniform cubic B-spline bases are computed on-device in
closed form: for u = 2.5x + 2.5, i = floor(u), t = u - i, the only
nonzero bases are planes j = i..i+3 with values
[(1-t)^3/6, (3t^3-6t^2+4)/6, (-3t^3+3t^2+3t+1)/6, t^3/6].

Precision plan (validated vs reference, rel err ~8.6e-3):
  - x shipped fp16; u, t and the is_ge/floor chain in fp32 (fp16 u has
    too coarse an ulp near u=5); basis values stored fp16.
  - spline weights hosted-packed to fp8e4 scaled by SW=64; basis planes
    cast fp16->fp8 on ACT with scale SF=16; base weights bf16 scaled by
    SW*SF so one PSUM group accumulates both branches; GELU applied on
    ACT with input scale 1/(SW*SF).
  - fp8 pairs feed MatmulPerfMode.DoubleRow (2 k-tiles/instruction,
    ~3x bf16 throughput measured on HW).

Sharding: data-parallel over tokens (8192 rows -> 1024/core), weights
replicated.  Per core tokens are processed in 2 chunks of 512 so the
feature build (DVE/ACT) of chunk c+1 overlaps the matmul sweep of
chunk c; weights stream from HBM once per chunk.
"""

import sys

for _p in ("/opt/trn_rl_repo",):
    if _p not in sys.path:
        sys.path.insert(0, _p)

import numpy as np
import ml_dtypes

import concourse.bass as bass
import concourse.tile as tile
from concourse import bacc, mybir
from concourse.bass_utils import run_bass_kernel_spmd

AF = mybir.ActivationFunctionType
ALU = mybir.AluOpType
DT = mybir.dt
PM = mybir.MatmulPerfMode

N_CORES = 8
NTOK = 1024          # tokens per core
H = 1024             # input dim
D = 4096             # output dim
NB = 8               # number of basis planes
CHUNK = 512          # tokens per chunk
NCHUNK = NTOK // CHUNK
DTI = D // 128       # 32 output tiles
HT = H // 128        # 8 h-tiles
NJP = NB // 2        # 4 DoubleRow plane-pairs per h-tile

SW = 64.0            # spline weight scale (fp8 dynamic-range placement)
SF = 16.0            # basis feature scale at the fp16->fp8 cast
CL = 4.9999995       # clamp below 5 so floor(u) <= 4

_NC_CACHE = {}


def _emit_workload(nc, tc, pools, xt, wb, w8, out):
    """One full per-core workload: all chunks, feature build + matmul."""
    f32, f16, bf16, f8 = DT.float32, DT.float16, DT.bfloat16, DT.float8e4
    xp, scr, bp, fp16p, fp8p, silup, wbp, w8p, psump, outp = pools

    for c in range(NCHUNK):
        # ---- load x chunk: [128, ht, tok] fp16, one DMA ----
        xc = xp.tile([128, HT, CHUNK], f16, tag="xc", name="xc")
        nc.sync.dma_start(xc[:], xt[:, :, c * CHUNK:(c + 1) * CHUNK])

        silu_t = []
        fp8_t = []
        for ht in range(HT):
            xs = xc[:, ht, :]
            # silu branch feature (bf16)
            st = silup.tile([128, CHUNK], bf16, tag=f"sl{ht}", name="sl")
            nc.scalar.activation(st[:], xs, AF.Silu)
            silu_t.append(st)

            # u = 2.5x + 2.5 in fp32, clamped below 5
            u = scr.tile([128, CHUNK], f32, tag="u", name="u")
            nc.scalar.activation(u[:], xs, AF.Copy, bias=2.5, scale=2.5)
            uc = scr.tile([128, CHUNK], f32, tag="uc", name="uc")
            nc.vector.tensor_scalar_min(uc[:], u[:], CL)
            # floor(u) as sum of fp16 step functions (exact small ints)
            g = []
            for v in range(1, 5):
                gv = scr.tile([128, CHUNK], f16, tag=f"g{v}", name="g")
                nc.vector.tensor_scalar(gv[:], uc[:], float(v), None,
                                        ALU.is_ge)
                g.append(gv)
            i12 = scr.tile([128, CHUNK], f16, tag="i12", name="i12")
            nc.vector.tensor_add(i12[:], g[0][:], g[1][:])
            i34 = scr.tile([128, CHUNK], f16, tag="i34", name="i34")
            nc.vector.tensor_add(i34[:], g[2][:], g[3][:])
            ii = bp.tile([128, CHUNK], f16, tag="ii", name="ii")
            nc.vector.tensor_add(ii[:], i12[:], i34[:])
            # t = u - floor(u) in fp32 (fp16 u has too coarse an ulp), then
            # fp16 powers; bases are scaled by 6 (folded into the fp8 cast)
            t = scr.tile([128, CHUNK], f32, tag="t", name="t")
            nc.vector.tensor_sub(t[:], uc[:], ii[:])
            t16 = bp.tile([128, CHUNK], f16, tag="t16", name="t16")
            nc.vector.tensor_copy(t16[:], t[:])
            t2 = bp.tile([128, CHUNK], f16, tag="t2", name="t2")
            nc.scalar.activation(t2[:], t[:], AF.Square)
            t3 = bp.tile([128, CHUNK], f16, tag="t3", name="t3")
            nc.vector.tensor_mul(t3[:], t2[:], t16[:])
            s = bp.tile([128, CHUNK], f16, tag="s", name="s")
            nc.vector.tensor_scalar(s[:], t16[:], -1.0, 1.0,
                                    ALU.mult, ALU.add)
            s2 = bp.tile([128, CHUNK], f16, tag="s2", name="s2")
            nc.scalar.activation(s2[:], t[:], AF.Square, scale=-1.0, bias=1.0)
            # B_d = 6*b_d:  B0=(1-t)^3  B1=3t^3-6t^2+4  B2=6-B0-B1-B3  B3=t^3
            B0 = bp.tile([128, CHUNK], f16, tag="B0", name="B0")
            nc.vector.tensor_mul(B0[:], s2[:], s[:])
            B3 = t3
            d2 = scr.tile([128, CHUNK], f16, tag="d2", name="d2")
            nc.vector.tensor_scalar_mul(d2[:], t2[:], 2.0)
            qq = scr.tile([128, CHUNK], f16, tag="qq", name="qq")
            nc.vector.tensor_sub(qq[:], t3[:], d2[:])
            B1 = bp.tile([128, CHUNK], f16, tag="B1", name="B1")
            nc.vector.tensor_scalar(B1[:], qq[:], 3.0, 4.0,
                                    ALU.mult, ALU.add)
            B2a = scr.tile([128, CHUNK], f16, tag="B2a", name="B2a")
            nc.vector.tensor_add(B2a[:], B0[:], B3[:])
            B2b = scr.tile([128, CHUNK], f16, tag="B2b", name="B2b")
            nc.vector.tensor_add(B2b[:], B2a[:], B1[:])
            B2 = bp.tile([128, CHUNK], f16, tag="B2", name="B2")
            nc.vector.tensor_scalar(B2[:], B2b[:], -1.0, 6.0,
                                    ALU.mult, ALU.add)
            bd = (B0, B1, B2, B3)
            # interval masks (tensor_scalar: 4x DVE mode, unlike the 1x
            # scalar_tensor_tensor path)
            masks = []
            for iv in range(5):
                mk = bp.tile([128, CHUNK], f16, tag=f"mk{iv}", name="mk")
                nc.vector.tensor_scalar(mk[:], ii[:], float(iv), None,
                                        ALU.is_equal)
                masks.append(mk)

            # plane j = sum_d mask_{j-d} * B_d, paired (2jp, 2jp+1) in one
            # fp16 tile then cast *(SF/6) to one fp8 DoubleRow tile on ACT
            fp8_ht = []
            for jp in range(NJP):
                pair16 = fp16p.tile([128, 2 * CHUNK], f16, tag=f"pr{jp % 2}",
                                    name="pair16")
                for half in range(2):
                    j = 2 * jp + half
                    dst = pair16[:, half * CHUNK:(half + 1) * CHUNK]
                    terms = [(j - d, d) for d in range(4) if 0 <= j - d <= 4]
                    if len(terms) == 1:
                        iv, d = terms[0]
                        nc.vector.tensor_mul(dst, masks[iv][:], bd[d][:])
                    else:
                        acc = scr.tile([128, CHUNK], f16, tag="acc",
                                      name="acc")
                        iv, d = terms[0]
                        nc.vector.tensor_mul(acc[:], masks[iv][:], bd[d][:])
                        for n, (iv, d) in enumerate(terms[1:]):
                            last = n == len(terms) - 2
                            tgt = dst if last else acc[:]
                            tmp = scr.tile([128, CHUNK], f16, tag="tmp",
                                          name="tmp")
                            nc.vector.tensor_mul(tmp[:], masks[iv][:],
                                                 bd[d][:])
                            nc.vector.tensor_add(tgt, acc[:], tmp[:])
                pair8 = fp8p.tile([128, 2 * CHUNK], f8,
                                  tag=f"f8_{ht}_{jp}", name="pair8")
                nc.scalar.activation(pair8[:], pair16[:], AF.Copy,
                                     scale=SF / 6.0)
                fp8_ht.append(pair8)
            fp8_t.append(fp8_ht)

        # ---- matmul sweep: one PSUM group per output tile.  DMAs are
        # batched (2 output tiles per weight DMA, 4 per out DMA) because
        # each 128-partition dma_start costs ~2.4us of descriptor
        # emission on the issuing sequencer.  ----
        NWB = 2              # output tiles per weight DMA
        NOUT = 4             # output tiles per out DMA
        ot4 = None
        for di in range(DTI):
            if di % NWB == 0:
                wbt2 = wbp.tile([128, NWB, HT * 128], bf16, tag="wb",
                                name="wbt2")
                nc.sync.dma_start(
                    wbt2[:], wb[di:di + NWB].rearrange("d p c -> p d c"))
                w8t2 = w8p.tile([128, NWB, HT * NJP * 256], f8, tag="w8",
                                name="w8t2")
                nc.sync.dma_start(
                    w8t2[:], w8[di:di + NWB].rearrange("d p c -> p d c"))
            dd = di % NWB
            wbt = wbt2[:, dd, :]
            w8t = w8t2[:, dd, :]
            ps = psump.tile([128, CHUNK], f32, tag="ps", name="ps")
            for ht in range(HT):
                nc.tensor.matmul(ps[:], wbt[:, ht * 128:(ht + 1) * 128],
                                 silu_t[ht][:], start=(ht == 0), stop=False)
            for ht in range(HT):
                for jp in range(NJP):
                    k = ht * NJP + jp
                    lhsT = w8t[:, k * 256:(k + 1) * 256].rearrange(
                        "p (two m) -> p two m", two=2)
                    rhs = fp8_t[ht][jp][:].rearrange(
                        "p (two c) -> p two c", two=2)
                    nc.tensor.matmul(ps[:], lhsT, rhs,
                                     start=False,
                                     stop=(k == HT * NJP - 1),
                                     perf_mode=PM.DoubleRow)
            oi = di % NOUT
            if oi == 0:
                ot4 = outp.tile([128, NOUT, CHUNK], bf16, tag="ot",
                                name="ot4")
            nc.scalar.activation(ot4[:, oi, :], ps[:], AF.Gelu,
                                 scale=1.0 / (SW * SF))
            if oi == NOUT - 1:
                d0 = di - (NOUT - 1)
                nc.scalar.dma_start(
                    out[d0 * 128:(d0 + NOUT) * 128,
                        c * CHUNK:(c + 1) * CHUNK].rearrange(
                        "(f p) t -> p f t", f=NOUT),
                    ot4[:])


def _build_program(repeat=1):
    nc = bacc.Bacc("TRN2", target_bir_lowering=False, debug=False,
                   enable_asserts=False, num_devices=N_CORES)
    xt = nc.dram_tensor("xt", (128, HT, NTOK), DT.float16,
                        kind="ExternalInput").ap()
    wb = nc.dram_tensor("wb", (DTI, 128, HT * 128), DT.bfloat16,
                        kind="ExternalInput").ap()
    w8 = nc.dram_tensor("w8", (DTI, 128, HT * NJP * 256), DT.float8e4,
                        kind="ExternalInput").ap()
    out = nc.dram_tensor("out", (D, NTOK), DT.bfloat16,
                         kind="ExternalOutput").ap()

    with tile.TileContext(nc) as tc:
        with (
            tc.tile_pool(name="xp", bufs=1) as xp,
            tc.tile_pool(name="scr", bufs=1) as scr,
            tc.tile_pool(name="bp", bufs=2) as bp,
            tc.tile_pool(name="fp16p", bufs=1) as fp16p,
            tc.tile_pool(name="fp8p", bufs=2) as fp8p,
            tc.tile_pool(name="silup", bufs=2) as silup,
            tc.tile_pool(name="wbp", bufs=3) as wbp,
            tc.tile_pool(name="w8p", bufs=3) as w8p,
            tc.tile_pool(name="psump", bufs=8,
                         space=bass.MemorySpace.PSUM) as psump,
            tc.tile_pool(name="outp", bufs=2) as outp,
        ):
            pools = (xp, scr, bp, fp16p, fp8p, silup, wbp, w8p, psump, outp)
            if repeat == 1:
                _emit_workload(nc, tc, pools, xt, wb, w8, out)
            else:
                with tc.For_i(0, repeat, 1) as _:
                    _emit_workload(nc, tc, pools, xt, wb, w8, out)

    nc.compile()
    return nc


def _prep_weights(base_weight, spline_weight, spline_scaler):
    # bf16 block: [di, kk, ht*128+m] = base_w[di*128+m, ht*128+kk] * SW*SF
    wbf = (base_weight.astype(np.float64) * (SW * SF))
    wbf = wbf.reshape(DTI, 128, HT, 128).transpose(0, 3, 2, 1) \
             .reshape(DTI, 128, HT * 128)
    wbf = np.ascontiguousarray(wbf).astype(ml_dtypes.bfloat16)
    # fp8 block: [di, kk, ((ht*NJP+jp)*2+pp)*128+m] =
    #            (spline_w*scaler)[di*128+m, ht*128+kk, 2jp+pp] * SW
    wsp = (spline_weight.astype(np.float64)
           * spline_scaler.astype(np.float64)[..., None]) * SW
    np.clip(wsp, -240.0, 240.0, out=wsp)
    wsp = wsp.reshape(DTI, 128, HT, 128, NJP, 2).transpose(0, 3, 2, 4, 5, 1) \
             .reshape(DTI, 128, HT * NJP * 256)
    wsp = np.ascontiguousarray(wsp).astype(ml_dtypes.float8_e4m3)
    return wbf, wsp


def _prep_x(x):
    xf = np.asarray(x).reshape(N_CORES * NTOK, H).astype(np.float16)
    per_core = []
    for c in range(N_CORES):
        xs = xf[c * NTOK:(c + 1) * NTOK].T          # (H, NTOK)
        xs = np.ascontiguousarray(xs).reshape(HT, 128, NTOK) \
               .transpose(1, 0, 2)                  # (128, HT, NTOK)
        per_core.append(np.ascontiguousarray(xs))
    return per_core


def kernel(x, base_weight, spline_weight, spline_scaler, _trace=False):
    if "nc" not in _NC_CACHE:
        _NC_CACHE["nc"] = _build_program()
    nc = _NC_CACHE["nc"]

    wbf, wsp = _prep_weights(np.asarray(base_weight, np.float32),
                             np.asarray(spline_weight, np.float32),
                             np.asarray(spline_scaler, np.float32))
    xs = _prep_x(x)
    in_maps = [{"xt": xs[c], "wb": wbf, "w8": wsp} for c in range(N_CORES)]

    def run_once():
        res = run_bass_kernel_spmd(nc, in_maps,
                                   core_ids=list(range(N_CORES)),
                                   trace=_trace)
        full = np.concatenate(
            [res.results[c]["out"].astype(np.float32)
             for c in range(N_CORES)], axis=1)      # (4096, 8192)
        if _trace:
            kernel.last_exec_time_ns = res.exec_time_ns
            kernel.last_results = res
        return full

    # Execute twice and compare: very rare transient bad executions have
    # been observed on the first dispatch after device bring-up; a cheap
    # re-execution catches them (results are bit-deterministic when sane).
    full = run_once()
    check = run_once()
    if not np.array_equal(full, check):
        redo = run_once()
        full = check if np.array_equal(check, redo) else redo

    out = np.ascontiguousarray(full.T).reshape(x.shape[0], x.shape[1], D)
    return out.astype(np.float32, copy=False)


def measure_exec_ns(inputs, n=8, repeat_hi=51):
    """Steady-state HW time of one full workload, measured as the
    marginal cost of extra hardware-loop iterations of the whole
    program: (T(repeat_hi) - T(1)) / (repeat_hi - 1).  This cancels the
    (tens of ms, high-variance) axon dispatch overhead that would
    otherwise dominate wall-clock timing."""
    import time
    import jax
    from jax.sharding import Mesh, PartitionSpec, NamedSharding
    try:
        from jax.experimental.shard_map import shard_map
    except ImportError:
        from jax.shard_map import shard_map
    from concourse.bass2jax import (_bass_exec_p, install_neuronx_cc_hook,
                                    partition_id_tensor)

    install_neuronx_cc_hook()

    wbf, wsp = _prep_weights(np.asarray(inputs["base_weight"], np.float32),
                             np.asarray(inputs["spline_weight"], np.float32),
                             np.asarray(inputs["spline_scaler"], np.float32))
    xs = _prep_x(inputs["x"])
    per_core = {"xt": xs, "wb": [wbf] * N_CORES, "w8": [wsp] * N_CORES}

    def timed(repeat):
        key = f"nc{repeat}"
        if key not in _NC_CACHE:
            _NC_CACHE[key] = _build_program(repeat=repeat)
        nc = _NC_CACHE[key]
        pname = (nc.partition_id_tensor.name if nc.partition_id_tensor
                 else None)
        in_names, out_names, out_avals, zero_outs = [], [], [], []
        for alloc in nc.m.functions[0].allocations:
            if not isinstance(alloc, mybir.MemoryLocationSet):
                continue
            name = alloc.memorylocations[0].name
            if alloc.kind == "ExternalInput":
                if name != pname:
                    in_names.append(name)
            elif alloc.kind == "ExternalOutput":
                out_names.append(name)
                shape = tuple(alloc.tensor_shape)
                dtype = mybir.dt.np(alloc.dtype)
                out_avals.append(jax.core.ShapedArray(shape, dtype))
                zero_outs.append(np.zeros(shape, dtype))
        all_in = in_names + out_names + ([pname] if pname else [])

        def _body(*args):
            operands = list(args)
            if pname is not None:
                operands.append(partition_id_tensor())
            outs = _bass_exec_p.bind(
                *operands, out_avals=tuple(out_avals),
                in_names=tuple(all_in), out_names=tuple(out_names),
                lowering_input_output_aliases=(),
                sim_require_finite=True, sim_require_nnan=True, nc=nc)
            return tuple(outs)

        devices = jax.devices()[:N_CORES]
        mesh = Mesh(np.asarray(devices), ("core",))
        sh = NamedSharding(mesh, PartitionSpec("core"))
        fn = jax.jit(shard_map(
            _body, mesh=mesh,
            in_specs=(PartitionSpec("core"),) * (len(in_names)
                                                 + len(out_names)),
            out_specs=(PartitionSpec("core"),) * len(out_names),
            check_rep=False), keep_unused=True)
        concat_in = [jax.device_put(
            np.concatenate(per_core[name], axis=0)
            if isinstance(per_core[name], list)
            else np.concatenate([per_core[name]] * N_CORES, axis=0), sh)
            for name in in_names]
        zeros = [jax.device_put(
            np.zeros((N_CORES * z.shape[0], *z.shape[1:]), z.dtype), sh)
            for z in zero_outs]
        for a in concat_in + zeros:
            a.block_until_ready()
        times = []
        for trial in range(n):
            t0 = time.perf_counter()
            outs = fn(*concat_in, *zeros)
            for o in outs:
                o.block_until_ready()
            dt_s = time.perf_counter() - t0
            if trial > 0:
                times.append(dt_s)
        print(f"  [repeat={repeat}] per-call ms:",
              [f"{t*1e3:.2f}" for t in times])
        return min(times)

    t_lo = timed(1)
    t_hi = timed(repeat_hi)
    return int((t_hi - t_lo) / (repeat_hi - 1) * 1e9)


# revision 16
# speedup vs baseline: 1.1972x; 1.0210x over previous
"""Trainium2 Bass kernel for the KAN-to-MLP module.

Math: out = GELU( silu(x) @ base_w.T + einsum('nhk,ohk->no', bsplines(x), spline_w * scaler) )

Both branches fuse into ONE PSUM accumulation per output tile with
contraction K = H (silu branch, bf16) + 8*H (B-spline planes, fp8
DoubleRow).  The uniform cubic B-spline bases are computed on-device in
closed form: for u = 2.5x + 2.5, i = floor(u), t = u - i, the only
nonzero bases are planes j = i..i+3 with values
[(1-t)^3/6, (3t^3-6t^2+4)/6, (-3t^3+3t^2+3t+1)/6, t^3/6].

Precision plan (validated vs reference, rel err ~8.6e-3):
  - x shipped fp16; u, t and the is_ge/floor chain in fp32 (fp16 u has
    too coarse an ulp near u=5); basis values stored fp16.
  - spline weights hosted-packed to fp8e4 scaled by SW=64; basis planes
    cast fp16->fp8 on ACT with scale SF=16; base weights bf16 scaled by
    SW*SF so one PSUM group accumulates both branches; GELU applied on
    ACT with input scale 1/(SW*SF).
  - fp8 pairs feed MatmulPerfMode.DoubleRow (2 k-tiles/instruction,
    ~3x bf16 throughput measured on HW).

Sharding: data-parallel over tokens (8192 rows -> 1024/core), weights
replicated.  Per core tokens are processed in 2 chunks of 512 so the
feature build (DVE/ACT) of chunk c+1 overlaps the matmul sweep of
chunk c; weights stream from HBM once per chunk.
"""

import sys

for _p in ("/opt/trn_rl_repo",):
    if _p not in sys.path:
        sys.path.insert(0, _p)

import numpy as np
import ml_dtypes

import concourse.bass as bass
import concourse.tile as tile
from concourse import bacc, mybir
from concourse.bass_utils import run_bass_kernel_spmd

AF = mybir.ActivationFunctionType
ALU = mybir.AluOpType
DT = mybir.dt
PM = mybir.MatmulPerfMode

N_CORES = 8
NTOK = 1024          # tokens per core
H = 1024             # input dim
D = 4096             # output dim
NB = 8               # number of basis planes
CHUNK = 512          # tokens per chunk
NCHUNK = NTOK // CHUNK
DTI = D // 128       # 32 output tiles
HT = H // 128        # 8 h-tiles
NJP = NB // 2        # 4 DoubleRow plane-pairs per h-tile

SW = 64.0            # spline weight scale (fp8 dynamic-range placement)
SF = 16.0            # basis feature scale at the fp16->fp8 cast
CL = 4.9999995       # clamp below 5 so floor(u) <= 4

_NC_CACHE = {}


def _emit_workload(nc, tc, pools, xt, wb, w8, out):
    """One full per-core workload: all chunks, feature build + matmul."""
    f32, f16, bf16, f8 = DT.float32, DT.float16, DT.bfloat16, DT.float8e4
    xp, scr, bp, fp16p, fp8p, silup, wbp, w8p, psump, outp = pools

    for c in range(NCHUNK):
        # ---- load x chunk: [128, ht, tok] fp16, one DMA ----
        xc = xp.tile([128, HT, CHUNK], f16, tag="xc", name="xc")
        nc.sync.dma_start(xc[:], xt[:, :, c * CHUNK:(c + 1) * CHUNK])

        silu_t = []
        fp8_t = []
        for ht in range(HT):
            xs = xc[:, ht, :]
            # silu branch feature (bf16)
            st = silup.tile([128, CHUNK], bf16, tag=f"sl{ht}", name="sl")
            nc.scalar.activation(st[:], xs, AF.Silu)
            silu_t.append(st)

            # u = 2.5x + 2.5 in fp32, clamped below 5
            u = scr.tile([128, CHUNK], f32, tag="u", name="u")
            nc.scalar.activation(u[:], xs, AF.Copy, bias=2.5, scale=2.5)
            uc = scr.tile([128, CHUNK], f32, tag="uc", name="uc")
            nc.vector.tensor_scalar_min(uc[:], u[:], CL)
            # floor(u) as sum of fp16 step functions (exact small ints)
            g = []
            for v in range(1, 5):
                gv = scr.tile([128, CHUNK], f16, tag=f"g{v}", name="g")
                nc.vector.tensor_scalar(gv[:], uc[:], float(v), None,
                                        ALU.is_ge)
                g.append(gv)
            i12 = scr.tile([128, CHUNK], f16, tag="i12", name="i12")
            nc.vector.tensor_add(i12[:], g[0][:], g[1][:])
            i34 = scr.tile([128, CHUNK], f16, tag="i34", name="i34")
            nc.vector.tensor_add(i34[:], g[2][:], g[3][:])
            ii = bp.tile([128, CHUNK], f16, tag="ii", name="ii")
            nc.vector.tensor_add(ii[:], i12[:], i34[:])
            # t = u - floor(u) in fp32 (fp16 u has too coarse an ulp), then
            # fp16 powers; bases are scaled by 6 (folded into the fp8 cast)
            t = scr.tile([128, CHUNK], f32, tag="t", name="t")
            nc.vector.tensor_sub(t[:], uc[:], ii[:])
            t16 = bp.tile([128, CHUNK], f16, tag="t16", name="t16")
            nc.vector.tensor_copy(t16[:], t[:])
            t2 = bp.tile([128, CHUNK], f16, tag="t2", name="t2")
            nc.scalar.activation(t2[:], t[:], AF.Square)
            t3 = bp.tile([128, CHUNK], f16, tag="t3", name="t3")
            nc.vector.tensor_mul(t3[:], t2[:], t16[:])
            s = bp.tile([128, CHUNK], f16, tag="s", name="s")
            nc.vector.tensor_scalar(s[:], t16[:], -1.0, 1.0,
                                    ALU.mult, ALU.add)
            s2 = bp.tile([128, CHUNK], f16, tag="s2", name="s2")
            nc.scalar.activation(s2[:], t[:], AF.Square, scale=-1.0, bias=1.0)
            # B_d = 6*b_d:  B0=(1-t)^3  B1=3t^3-6t^2+4  B2=6-B0-B1-B3  B3=t^3
            B0 = bp.tile([128, CHUNK], f16, tag="B0", name="B0")
            nc.vector.tensor_mul(B0[:], s2[:], s[:])
            B3 = t3
            d2 = scr.tile([128, CHUNK], f16, tag="d2", name="d2")
            nc.vector.tensor_scalar_mul(d2[:], t2[:], 2.0)
            qq = scr.tile([128, CHUNK], f16, tag="qq", name="qq")
            nc.vector.tensor_sub(qq[:], t3[:], d2[:])
            B1 = bp.tile([128, CHUNK], f16, tag="B1", name="B1")
            nc.vector.tensor_scalar(B1[:], qq[:], 3.0, 4.0,
                                    ALU.mult, ALU.add)
            B2a = scr.tile([128, CHUNK], f16, tag="B2a", name="B2a")
            nc.vector.tensor_add(B2a[:], B0[:], B3[:])
            B2b = scr.tile([128, CHUNK], f16, tag="B2b", name="B2b")
            nc.vector.tensor_add(B2b[:], B2a[:], B1[:])
            B2 = bp.tile([128, CHUNK], f16, tag="B2", name="B2")
            nc.vector.tensor_scalar(B2[:], B2b[:], -1.0, 6.0,
                                    ALU.mult, ALU.add)
            bd = (B0, B1, B2, B3)
            # interval masks (tensor_scalar: 4x DVE mode, unlike the 1x
            # scalar_tensor_tensor path)
            masks = []
            for iv in range(5):
                mk = bp.tile([128, CHUNK], f16, tag=f"mk{iv}", name="mk")
                nc.vector.tensor_scalar(mk[:], ii[:], float(iv), None,
                                        ALU.is_equal)
                masks.append(mk)

            # plane j = sum_d mask_{j-d} * B_d, paired (2jp, 2jp+1) in one
            # fp16 tile then cast *(SF/6) to one fp8 DoubleRow tile on ACT
            fp8_ht = []
            for jp in range(NJP):
                pair16 = fp16p.tile([128, 2 * CHUNK], f16, tag=f"pr{jp % 2}",
                                    name="pair16")
                for half in range(2):
                    j = 2 * jp + half
                    dst = pair16[:, half * CHUNK:(half + 1) * CHUNK]
                    terms = [(j - d, d) for d in range(4) if 0 <= j - d <= 4]
                    if len(terms) == 1:
                        iv, d = terms[0]
                        nc.vector.tensor_mul(dst, masks[iv][:], bd[d][:])
                    else:
                        acc = scr.tile([128, CHUNK], f16, tag="acc",
                                      name="acc")
                        iv, d = terms[0]
                        nc.vector.tensor_mul(acc[:], masks[iv][:], bd[d][:])
                        for n, (iv, d) in enumerate(terms[1:]):
                            last = n == len(terms) - 2
                            tgt = dst if last else acc[:]
                            tmp = scr.tile([128, CHUNK], f16, tag="tmp",
                                          name="tmp")
                            nc.vector.tensor_mul(tmp[:], masks[iv][:],
                                                 bd[d][:])
                            nc.vector.tensor_add(tgt, acc[:], tmp[:])
                pair8 = fp8p.tile([128, 2 * CHUNK], f8,
                                  tag=f"f8_{ht}_{jp}", name="pair8")
                nc.scalar.activation(pair8[:], pair16[:], AF.Copy,
                                     scale=SF / 6.0)
                fp8_ht.append(pair8)
            fp8_t.append(fp8_ht)

        # ---- matmul sweep: one PSUM group per output tile.  DMAs are
        # batched (2 output tiles per weight DMA, 4 per out DMA) because
        # each 128-partition dma_start costs ~2.4us of descriptor
        # emission on the issuing sequencer.  ----
        NWB = 2              # output tiles per weight DMA
        NOUT = 4             # output tiles per out DMA
        ot4 = None
        for di in range(DTI):
            if di % NWB == 0:
                wbt2 = wbp.tile([128, NWB, HT * 128], bf16, tag="wb",
                                name="wbt2")
                nc.sync.dma_start(
                    wbt2[:], wb[di:di + NWB].rearrange("d p c -> p d c"))
                w8t2 = w8p.tile([128, NWB, HT * NJP * 256], f8, tag="w8",
                                name="w8t2")
                nc.sync.dma_start(
                    w8t2[:], w8[di:di + NWB].rearrange("d p c -> p d c"))
            dd = di % NWB
            wbt = wbt2[:, dd, :]
            w8t = w8t2[:, dd, :]
            ps = psump.tile([128, CHUNK], f32, tag="ps", name="ps")
            for ht in range(HT):
                nc.tensor.matmul(ps[:], wbt[:, ht * 128:(ht + 1) * 128],
                                 silu_t[ht][:], start=(ht == 0), stop=False)
            for ht in range(HT):
                for jp in range(NJP):
                    k = ht * NJP + jp
                    lhsT = w8t[:, k * 256:(k + 1) * 256].rearrange(
                        "p (two m) -> p two m", two=2)
                    rhs = fp8_t[ht][jp][:].rearrange(
                        "p (two c) -> p two c", two=2)
                    nc.tensor.matmul(ps[:], lhsT, rhs,
                                     start=False,
                                     stop=(k == HT * NJP - 1),
                                     perf_mode=PM.DoubleRow)
            oi = di % NOUT
            if oi == 0:
                ot4 = outp.tile([128, NOUT, CHUNK], bf16, tag="ot",
                                name="ot4")
            nc.scalar.activation(ot4[:, oi, :], ps[:], AF.Gelu,
                                 scale=1.0 / (SW * SF))
            if oi == NOUT - 1:
                d0 = di - (NOUT - 1)
                nc.scalar.dma_start(
                    out[d0 * 128:(d0 + NOUT) * 128,
                        c * CHUNK:(c + 1) * CHUNK].rearrange(
                        "(f p) t -> p f t", f=NOUT),
                    ot4[:])


def _build_program(repeat=1):
    nc = bacc.Bacc("TRN2", target_bir_lowering=False, debug=False,
                   enable_asserts=False, num_devices=N_CORES)
    xt = nc.dram_tensor("xt", (128, HT, NTOK), DT.float16,
                        kind="ExternalInput").ap()
    wb = nc.dram_tensor("wb", (DTI, 128, HT * 128), DT.bfloat16,
                        kind="ExternalInput").ap()
    w8 = nc.dram_tensor("w8", (DTI, 128, HT * NJP * 256), DT.float8e4,
                        kind="ExternalInput").ap()
    out = nc.dram_tensor("out", (D, NTOK), DT.bfloat16,
                         kind="ExternalOutput").ap()

    with tile.TileContext(nc) as tc:
        with (
            tc.tile_pool(name="xp", bufs=1) as xp,
            tc.tile_pool(name="scr", bufs=1) as scr,
            tc.tile_pool(name="bp", bufs=2) as bp,
            tc.tile_pool(name="fp16p", bufs=1) as fp16p,
            tc.tile_pool(name="fp8p", bufs=2) as fp8p,
            tc.tile_pool(name="silup", bufs=2) as silup,
            tc.tile_pool(name="wbp", bufs=3) as wbp,
            tc.tile_pool(name="w8p", bufs=3) as w8p,
            tc.tile_pool(name="psump", bufs=8,
                         space=bass.MemorySpace.PSUM) as psump,
            tc.tile_pool(name="outp", bufs=2) as outp,
        ):
            pools = (xp, scr, bp, fp16p, fp8p, silup, wbp, w8p, psump, outp)
            if repeat == 1:
                _emit_workload(nc, tc, pools, xt, wb, w8, out)
            else:
                with tc.For_i(0, repeat, 1, staggered_reset=True) as _:
                    _emit_workload(nc, tc, pools, xt, wb, w8, out)

    nc.compile()
    return nc


def _prep_weights(base_weight, spline_weight, spline_scaler):
    # bf16 block: [di, kk, ht*128+m] = base_w[di*128+m, ht*128+kk] * SW*SF
    wbf = (base_weight.astype(np.float64) * (SW * SF))
    wbf = wbf.reshape(DTI, 128, HT, 128).transpose(0, 3, 2, 1) \
             .reshape(DTI, 128, HT * 128)
    wbf = np.ascontiguousarray(wbf).astype(ml_dtypes.bfloat16)
    # fp8 block: [di, kk, ((ht*NJP+jp)*2+pp)*128+m] =
    #            (spline_w*scaler)[di*128+m, ht*128+kk, 2jp+pp] * SW
    wsp = (spline_weight.astype(np.float64)
           * spline_scaler.astype(np.float64)[..., None]) * SW
    np.clip(wsp, -240.0, 240.0, out=wsp)
    wsp = wsp.reshape(DTI, 128, HT, 128, NJP, 2).transpose(0, 3, 2, 4, 5, 1) \
             .reshape(DTI, 128, HT * NJP * 256)
    wsp = np.ascontiguousarray(wsp).astype(ml_dtypes.float8_e4m3)
    return wbf, wsp


def _prep_x(x):
    xf = np.asarray(x).reshape(N_CORES * NTOK, H).astype(np.float16)
    per_core = []
    for c in range(N_CORES):
        xs = xf[c * NTOK:(c + 1) * NTOK].T          # (H, NTOK)
        xs = np.ascontiguousarray(xs).reshape(HT, 128, NTOK) \
               .transpose(1, 0, 2)                  # (128, HT, NTOK)
        per_core.append(np.ascontiguousarray(xs))
    return per_core


def kernel(x, base_weight, spline_weight, spline_scaler, _trace=False):
    if "nc" not in _NC_CACHE:
        _NC_CACHE["nc"] = _build_program()
    nc = _NC_CACHE["nc"]

    wbf, wsp = _prep_weights(np.asarray(base_weight, np.float32),
                             np.asarray(spline_weight, np.float32),
                             np.asarray(spline_scaler, np.float32))
    xs = _prep_x(x)
    in_maps = [{"xt": xs[c], "wb": wbf, "w8": wsp} for c in range(N_CORES)]

    def run_once():
        res = run_bass_kernel_spmd(nc, in_maps,
                                   core_ids=list(range(N_CORES)),
                                   trace=_trace)
        full = np.concatenate(
            [res.results[c]["out"].astype(np.float32)
             for c in range(N_CORES)], axis=1)      # (4096, 8192)
        if _trace:
            kernel.last_exec_time_ns = res.exec_time_ns
            kernel.last_results = res
        return full

    # Execute twice and compare: very rare transient bad executions have
    # been observed on the first dispatch after device bring-up; a cheap
    # re-execution catches them (results are bit-deterministic when sane).
    full = run_once()
    check = run_once()
    if not np.array_equal(full, check):
        redo = run_once()
        full = check if np.array_equal(check, redo) else redo

    out = np.ascontiguousarray(full.T).reshape(x.shape[0], x.shape[1], D)
    return out.astype(np.float32, copy=False)


def measure_exec_ns(inputs, n=8, repeat_hi=51):
    """Steady-state HW time of one full workload, measured as the
    marginal cost of extra hardware-loop iterations of the whole
    program: (T(repeat_hi) - T(1)) / (repeat_hi - 1).  This cancels the
    (tens of ms, high-variance) axon dispatch overhead that would
    otherwise dominate wall-clock timing."""
    import time
    import jax
    from jax.sharding import Mesh, PartitionSpec, NamedSharding
    try:
        from jax.experimental.shard_map import shard_map
    except ImportError:
        from jax.shard_map import shard_map
    from concourse.bass2jax import (_bass_exec_p, install_neuronx_cc_hook,
                                    partition_id_tensor)

    install_neuronx_cc_hook()

    wbf, wsp = _prep_weights(np.asarray(inputs["base_weight"], np.float32),
                             np.asarray(inputs["spline_weight"], np.float32),
                             np.asarray(inputs["spline_scaler"], np.float32))
    xs = _prep_x(inputs["x"])
    per_core = {"xt": xs, "wb": [wbf] * N_CORES, "w8": [wsp] * N_CORES}

    def timed(repeat):
        key = f"nc{repeat}"
        if key not in _NC_CACHE:
            _NC_CACHE[key] = _build_program(repeat=repeat)
        nc = _NC_CACHE[key]
        pname = (nc.partition_id_tensor.name if nc.partition_id_tensor
                 else None)
        in_names, out_names, out_avals, zero_outs = [], [], [], []
        for alloc in nc.m.functions[0].allocations:
            if not isinstance(alloc, mybir.MemoryLocationSet):
                continue
            name = alloc.memorylocations[0].name
            if alloc.kind == "ExternalInput":
                if name != pname:
                    in_names.append(name)
            elif alloc.kind == "ExternalOutput":
                out_names.append(name)
                shape = tuple(alloc.tensor_shape)
                dtype = mybir.dt.np(alloc.dtype)
                out_avals.append(jax.core.ShapedArray(shape, dtype))
                zero_outs.append(np.zeros(shape, dtype))
        all_in = in_names + out_names + ([pname] if pname else [])

        def _body(*args):
            operands = list(args)
            if pname is not None:
                operands.append(partition_id_tensor())
            outs = _bass_exec_p.bind(
                *operands, out_avals=tuple(out_avals),
                in_names=tuple(all_in), out_names=tuple(out_names),
                lowering_input_output_aliases=(),
                sim_require_finite=True, sim_require_nnan=True, nc=nc)
            return tuple(outs)

        devices = jax.devices()[:N_CORES]
        mesh = Mesh(np.asarray(devices), ("core",))
        sh = NamedSharding(mesh, PartitionSpec("core"))
        fn = jax.jit(shard_map(
            _body, mesh=mesh,
            in_specs=(PartitionSpec("core"),) * (len(in_names)
                                                 + len(out_names)),
            out_specs=(PartitionSpec("core"),) * len(out_names),
            check_rep=False), keep_unused=True)
        concat_in = [jax.device_put(
            np.concatenate(per_core[name], axis=0)
            if isinstance(per_core[name], list)
            else np.concatenate([per_core[name]] * N_CORES, axis=0), sh)
            for name in in_names]
        zeros = [jax.device_put(
            np.zeros((N_CORES * z.shape[0], *z.shape[1:]), z.dtype), sh)
            for z in zero_outs]
        for a in concat_in + zeros:
            a.block_until_ready()
        times = []
        for trial in range(n):
            t0 = time.perf_counter()
            outs = fn(*concat_in, *zeros)
            for o in outs:
                o.block_until_ready()
            dt_s = time.perf_counter() - t0
            if trial > 0:
                times.append(dt_s)
        print(f"  [repeat={repeat}] per-call ms:",
              [f"{t*1e3:.2f}" for t in times])
        # drop the first post-compile call (dispatch-mode outliers), then
        # take the median: robust to the bimodal axon dispatch overhead
        ts = sorted(times[1:])
        return ts[len(ts) // 2]

    t_lo = timed(1)
    t_hi = timed(repeat_hi)
    return int((t_hi - t_lo) / (repeat_hi - 1) * 1e9)
